# revision 3
# baseline (speedup 1.0000x reference)
"""Data-parallel 4-layer MLP (795->512->256->128->14, ELU) for 8 trn2 cores.

Strategy: shard batch (65536 -> 8 x 8192). Weights replicated. All activations
live in transposed layout [features, batch] so every matmul contracts over the
partition dim with weights stationary. The host pre-transposes x per shard and
pre-transposes weights, so the kernel does zero on-chip transposes.

ELU is computed as ELU(z)+1 = relu(z) + exp(min(z,0)); the +1 shift is folded
into the next layer's bias on the host (b' = b - W @ ones), so no on-chip -1.

Per 512-column chunk: L1 = 28 matmuls (K=7 tiles, M=4), L2 = 8, L3 = 2, L4 = 1.
Emission is software-pipelined 4 deep (L1 of chunk t, L2 of t-1, L3 of t-2,
L4 of t-3) so the PE never waits on an ELU chain.
"""

import numpy as np

import concourse.bass as bass
import concourse.mybir as mybir
import concourse.tile as tile
from concourse import bacc
from concourse.bass_utils import run_bass_kernel_spmd

F32 = mybir.dt.float32
AF = mybir.ActivationFunctionType
ALU = mybir.AluOpType

N_CORES = 8
B = 65536
BS = B // N_CORES  # 8192 rows per core
NB = 512           # batch columns per chunk (= one fp32 PSUM bank)
NCHUNK = BS // NB  # 16
DIMS = [795, 512, 256, 128, 14]

# K-tiling of the layer-1 contraction dim (795 = 6*128 + 27)
K1 = [(k * 128, min(128, DIMS[0] - k * 128)) for k in range((DIMS[0] + 127) // 128)]

_CACHE = {}


def _build_nc():
    if "nc" in _CACHE:
        return _CACHE["nc"]
    nc = bacc.Bacc(None, target_bir_lowering=False)

    xt_d = nc.dram_tensor("xt", [DIMS[0], BS], F32, kind="ExternalInput")
    w1_d = nc.dram_tensor("w1t", [DIMS[0], DIMS[1]], F32, kind="ExternalInput")
    w2_d = nc.dram_tensor("w2t", [DIMS[1], DIMS[2]], F32, kind="ExternalInput")
    w3_d = nc.dram_tensor("w3t", [DIMS[2], DIMS[3]], F32, kind="ExternalInput")
    w4_d = nc.dram_tensor("w4t", [DIMS[3], DIMS[4]], F32, kind="ExternalInput")
    b1_d = nc.dram_tensor("b1c", [128, 4], F32, kind="ExternalInput")
    b2_d = nc.dram_tensor("b2c", [128, 2], F32, kind="ExternalInput")
    b3_d = nc.dram_tensor("b3c", [128, 1], F32, kind="ExternalInput")
    b4_d = nc.dram_tensor("b4c", [DIMS[4], 1], F32, kind="ExternalInput")
    yt_d = nc.dram_tensor("yt", [DIMS[4], BS], F32, kind="ExternalOutput")

    with tile.TileContext(nc) as tc:
        with (
            tc.tile_pool(name="wpool", bufs=1) as wpool,
            tc.tile_pool(name="cpool", bufs=1) as cpool,
            tc.tile_pool(name="xpool", bufs=3) as xpool,
            tc.tile_pool(name="hpool", bufs=2) as hpool,
            tc.tile_pool(name="tpool", bufs=4) as tpool,
            tc.tile_pool(name="opool", bufs=3) as opool,
            tc.tile_pool(name="psum", bufs=1, space=bass.MemorySpace.PSUM) as pp,
        ):
            # ---- resident weights + biases ----
            w1 = []
            for k, (k0, kp) in enumerate(K1):
                t = wpool.tile([kp, DIMS[1]], F32, tag=f"w1k{k}")
                nc.sync.dma_start(t[:], w1_d[k0 : k0 + kp, :])
                w1.append(t)
            w2 = []
            for k in range(4):
                t = wpool.tile([128, DIMS[2]], F32, tag=f"w2k{k}")
                nc.sync.dma_start(t[:], w2_d[k * 128 : (k + 1) * 128, :])
                w2.append(t)
            w3 = []
            for k in range(2):
                t = wpool.tile([128, DIMS[3]], F32, tag=f"w3k{k}")
                nc.sync.dma_start(t[:], w3_d[k * 128 : (k + 1) * 128, :])
                w3.append(t)
            w4 = wpool.tile([128, DIMS[4]], F32, tag="w4")
            nc.sync.dma_start(w4[:], w4_d[:, :])

            b1t = cpool.tile([128, 4], F32, tag="b1")
            nc.sync.dma_start(b1t[:], b1_d[:, :])
            b2t = cpool.tile([128, 2], F32, tag="b2")
            nc.sync.dma_start(b2t[:], b2_d[:, :])
            b3t = cpool.tile([128, 1], F32, tag="b3")
            nc.sync.dma_start(b3t[:], b3_d[:, :])
            b4t = cpool.tile([DIMS[4], 1], F32, tag="b4")
            nc.sync.dma_start(b4t[:], b4_d[:, :])

            xs = {}   # chunk -> list of 7 x-tiles
            h1s = {}  # chunk -> 4 tiles
            h2s = {}  # chunk -> 2 tiles
            h3s = {}  # chunk -> 1 tile

            def elu_plus1(ps, bias_ap, h):
                """h = relu(ps+b) + exp(min(ps+b, 0)) into SBUF tile h."""
                zn = tpool.tile([ps.shape[0], NB], F32, tag="zn")
                ex = tpool.tile([ps.shape[0], NB], F32, tag="ex")
                nc.scalar.activation(h[:], ps[:], AF.Relu, bias=bias_ap)
                nc.vector.tensor_scalar(zn[:], ps[:], bias_ap, 0.0, ALU.add, ALU.min)
                nc.scalar.activation(ex[:], zn[:], AF.Exp)
                nc.gpsimd.tensor_tensor(h[:], h[:], ex[:], ALU.add)

            def emit_dma_x(t):
                c0 = t * NB
                tiles = []
                for k, (k0, kp) in enumerate(K1):
                    xt = xpool.tile([kp, NB], F32, tag=f"x{k}")
                    nc.sync.dma_start(xt[:], xt_d[k0 : k0 + kp, c0 : c0 + NB])
                    tiles.append(xt)
                xs[t] = tiles

            def emit_l1(t):
                xts = xs.pop(t)
                hs = []
                for m in range(4):
                    ps = pp.tile([128, NB], F32, tag=f"ps1m{m}")
                    for k in range(len(K1)):
                        nc.tensor.matmul(
                            ps[:],
                            w1[k][:, m * 128 : (m + 1) * 128],
                            xts[k][:],
                            start=(k == 0),
                            stop=(k == len(K1) - 1),
                        )
                    h = hpool.tile([128, NB], F32, tag=f"h1m{m}")
                    elu_plus1(ps, b1t[:, m : m + 1], h)
                    hs.append(h)
                h1s[t] = hs

            def emit_l2(t):
                hin = h1s.pop(t)
                hs = []
                for m in range(2):
                    ps = pp.tile([128, NB], F32, tag=f"ps2m{m}")
                    for k in range(4):
                        nc.tensor.matmul(
                            ps[:],
                            w2[k][:, m * 128 : (m + 1) * 128],
                            hin[k][:],
                            start=(k == 0),
                            stop=(k == 3),
                        )
                    h = hpool.tile([128, NB], F32, tag=f"h2m{m}")
                    elu_plus1(ps, b2t[:, m : m + 1], h)
                    hs.append(h)
                h2s[t] = hs

            def emit_l3(t):
                hin = h2s.pop(t)
                ps = pp.tile([128, NB], F32, tag="ps3")
                for k in range(2):
                    nc.tensor.matmul(
                        ps[:], w3[k][:, :], hin[k][:], start=(k == 0), stop=(k == 1)
                    )
                h = hpool.tile([128, NB], F32, tag="h3")
                elu_plus1(ps, b3t[:, 0:1], h)
                h3s[t] = h

            def emit_l4(t):
                hin = h3s.pop(t)
                c0 = t * NB
                ps = pp.tile([DIMS[4], NB], F32, tag="ps4")
                nc.tensor.matmul(ps[:], w4[:, :], hin[:], start=True, stop=True)
                ot = opool.tile([DIMS[4], NB], F32, tag="yt")
                nc.scalar.activation(ot[:], ps[:], AF.Identity, bias=b4t[:, 0:1])
                nc.sync.dma_start(yt_d[:, c0 : c0 + NB], ot[:])

            # ---- software-pipelined emission, 4 deep ----
            emit_dma_x(0)
            for t in range(NCHUNK + 3):
                if t < NCHUNK:
                    if t + 1 < NCHUNK:
                        emit_dma_x(t + 1)
                    emit_l1(t)
                if 0 <= t - 1 < NCHUNK:
                    emit_l2(t - 1)
                if 0 <= t - 2 < NCHUNK:
                    emit_l3(t - 2)
                if 0 <= t - 3 < NCHUNK:
                    emit_l4(t - 3)

    nc.compile()
    _CACHE["nc"] = nc
    return nc


def _prep_in_maps(x, W1, b1, W2, b2, W3, b3, W4, b4):
    x = np.asarray(x, dtype=np.float32)
    shared = {
        "w1t": np.ascontiguousarray(np.asarray(W1, np.float32).T),
        "w2t": np.ascontiguousarray(np.asarray(W2, np.float32).T),
        "w3t": np.ascontiguousarray(np.asarray(W3, np.float32).T),
        "w4t": np.ascontiguousarray(np.asarray(W4, np.float32).T),
    }
    # bias adjustments: downstream layers see h' = elu(h)+1, so subtract W @ 1.
    b1a = np.asarray(b1, np.float32)
    b2a = np.asarray(b2, np.float32) - np.asarray(W2, np.float32).sum(axis=1)
    b3a = np.asarray(b3, np.float32) - np.asarray(W3, np.float32).sum(axis=1)
    b4a = np.asarray(b4, np.float32) - np.asarray(W4, np.float32).sum(axis=1)
    shared["b1c"] = np.ascontiguousarray(b1a.reshape(4, 128).T)
    shared["b2c"] = np.ascontiguousarray(b2a.reshape(2, 128).T)
    shared["b3c"] = np.ascontiguousarray(b3a.reshape(1, 128).T)
    shared["b4c"] = np.ascontiguousarray(b4a.reshape(DIMS[4], 1))

    in_maps = []
    for i in range(N_CORES):
        xs = np.ascontiguousarray(x[i * BS : (i + 1) * BS].T)  # [795, 8192]
        in_maps.append({**shared, "xt": xs})
    return in_maps


def _run(in_maps, trace=False, **kw):
    nc = _build_nc()
    return run_bass_kernel_spmd(nc, in_maps, list(range(N_CORES)), trace=trace, **kw)


def kernel(x, W1, b1, W2, b2, W3, b3, W4, b4):
    in_maps = _prep_in_maps(x, W1, b1, W2, b2, W3, b3, W4, b4)
    res = _run(in_maps).results
    y = np.empty((B, DIMS[4]), dtype=np.float32)
    for i in range(N_CORES):
        y[i * BS : (i + 1) * BS, :] = res[i]["yt"].T
    return y


# revision 15
# speedup vs baseline: 2.9040x; 2.9040x over previous
"""Data-parallel 4-layer MLP (795->512->256->128->14, ELU) for 8 trn2 cores.

Strategy: shard batch (65536 -> 8 x 8192). Weights replicated. All activations
live in transposed layout [features, batch] so every matmul contracts over the
partition dim with weights stationary. The host pre-transposes x per shard and
pre-transposes weights, so the kernel does zero on-chip transposes.

ELU is computed as ELU(z)+1 = relu(z) + exp(min(z,0)); the +1 shift is folded
into the next layer's bias on the host (b' = b - W @ ones), so no on-chip -1.

Per 512-column chunk: L1 = 28 matmuls (K=7 tiles, M=4), L2 = 8, L3 = 2, L4 = 1.
Emission is software-pipelined 4 deep (L1 of chunk t, L2 of t-1, L3 of t-2,
L4 of t-3) so the PE never waits on an ELU chain.
"""

import numpy as np

import concourse.bass as bass
import concourse.mybir as mybir
import concourse.tile as tile
from concourse import bacc
from concourse.bass_utils import run_bass_kernel_spmd

F32 = mybir.dt.float32
F32R = mybir.dt.float32r  # fp32 "replicated": full-rate PE (1 cyc/row at N>=256)
AF = mybir.ActivationFunctionType
ALU = mybir.AluOpType

N_CORES = 8
B = 65536
BS = B // N_CORES  # 8192 rows per core
NB = 512           # batch columns per chunk (= one fp32 PSUM bank)
NCHUNK = BS // NB  # 16
DIMS = [795, 512, 256, 128, 14]

# K-tiling of the layer-1 contraction dim (795 = 6*128 + 27)
K1 = [(k * 128, min(128, DIMS[0] - k * 128)) for k in range((DIMS[0] + 127) // 128)]

_CACHE = {}


def _build_nc():
    if "nc" in _CACHE:
        return _CACHE["nc"]
    nc = bacc.Bacc(None, target_bir_lowering=False)

    xt_d = nc.dram_tensor("xt", [DIMS[0], BS], F32R, kind="ExternalInput")
    w1_d = nc.dram_tensor("w1t", [DIMS[0], DIMS[1]], F32R, kind="ExternalInput")
    w2_d = nc.dram_tensor("w2t", [DIMS[1], DIMS[2]], F32R, kind="ExternalInput")
    w3_d = nc.dram_tensor("w3t", [DIMS[2], DIMS[3]], F32R, kind="ExternalInput")
    w4_d = nc.dram_tensor("w4t", [DIMS[3], DIMS[4]], F32R, kind="ExternalInput")
    b1_d = nc.dram_tensor("b1c", [128, 4], F32, kind="ExternalInput")
    b2_d = nc.dram_tensor("b2c", [128, 2], F32, kind="ExternalInput")
    b3_d = nc.dram_tensor("b3c", [128, 1], F32, kind="ExternalInput")
    b4_d = nc.dram_tensor("b4c", [DIMS[4], 1], F32, kind="ExternalInput")
    yt_d = nc.dram_tensor("yt", [DIMS[4], BS], F32, kind="ExternalOutput")

    with tile.TileContext(nc) as tc:
        with (
            tc.tile_pool(name="wpool", bufs=1) as wpool,
            tc.tile_pool(name="cpool", bufs=1) as cpool,
            tc.tile_pool(name="xpool", bufs=3) as xpool,
            tc.tile_pool(name="hpool", bufs=2) as hpool,
            tc.tile_pool(name="tpool", bufs=4) as tpool,
            tc.tile_pool(name="opool", bufs=3) as opool,
            tc.tile_pool(name="psum", bufs=1, space=bass.MemorySpace.PSUM) as pp,
        ):
            # ---- resident weights + biases ----
            w1 = []
            for k, (k0, kp) in enumerate(K1):
                t = wpool.tile([kp, DIMS[1]], F32R, tag=f"w1k{k}")
                nc.sync.dma_start(t[:], w1_d[k0 : k0 + kp, :])
                w1.append(t)
            w2 = []
            for k in range(4):
                t = wpool.tile([128, DIMS[2]], F32R, tag=f"w2k{k}")
                nc.sync.dma_start(t[:], w2_d[k * 128 : (k + 1) * 128, :])
                w2.append(t)
            w3 = []
            for k in range(2):
                t = wpool.tile([128, DIMS[3]], F32R, tag=f"w3k{k}")
                nc.sync.dma_start(t[:], w3_d[k * 128 : (k + 1) * 128, :])
                w3.append(t)
            w4 = wpool.tile([128, DIMS[4]], F32R, tag="w4")
            nc.sync.dma_start(w4[:], w4_d[:, :])

            b1t = cpool.tile([128, 4], F32, tag="b1")
            nc.sync.dma_start(b1t[:], b1_d[:, :])
            b2t = cpool.tile([128, 2], F32, tag="b2")
            nc.sync.dma_start(b2t[:], b2_d[:, :])
            b3t = cpool.tile([128, 1], F32, tag="b3")
            nc.sync.dma_start(b3t[:], b3_d[:, :])
            b4t = cpool.tile([DIMS[4], 1], F32, tag="b4")
            nc.sync.dma_start(b4t[:], b4_d[:, :])

            xs = {}   # chunk -> list of 7 x-tiles
            h1s = {}  # chunk -> 4 tiles
            h2s = {}  # chunk -> 2 tiles
            h3s = {}  # chunk -> 1 tile

            def elu_plus1(ps, bias_ap, h, relu_on_act):
                """h = elu(z)+1 = relu(z) + min(exp(z), 1), z = ps + bias.

                exp(z) is safe unclamped here: |z| stays O(10) for this net,
                far from fp32 overflow, and positive-z lanes are replaced by
                the min with 1. The relu pass is split between ACT and DVE
                to balance engine load (both can read PSUM; GPSIMD cannot,
                and Pool rejects scalar_tensor_tensor)."""
                ex = tpool.tile([ps.shape[0], NB], F32, tag="ex")
                zp = tpool.tile([ps.shape[0], NB], F32, tag="zp")
                nc.scalar.activation(ex[:], ps[:], AF.Exp, bias=bias_ap)
                if relu_on_act:
                    nc.scalar.activation(zp[:], ps[:], AF.Relu, bias=bias_ap)
                else:
                    nc.vector.tensor_scalar(
                        zp[:], ps[:], bias_ap, 0.0, ALU.add, ALU.max
                    )
                nc.vector.scalar_tensor_tensor(
                    h[:], ex[:], 1.0, zp[:], ALU.min, ALU.add
                )

            def emit_dma_x(t):
                c0 = t * NB
                tiles = []
                for k, (k0, kp) in enumerate(K1):
                    xt = xpool.tile([kp, NB], F32R, tag=f"x{k}")
                    nc.sync.dma_start(xt[:], xt_d[k0 : k0 + kp, c0 : c0 + NB])
                    tiles.append(xt)
                xs[t] = tiles

            def emit_l1(t):
                xts = xs.pop(t)
                hs = []
                for m in range(4):
                    ps = pp.tile([128, NB], F32, tag=f"ps1m{m}")
                    for k in range(len(K1)):
                        nc.tensor.matmul(
                            ps[:],
                            w1[k][:, m * 128 : (m + 1) * 128],
                            xts[k][:],
                            start=(k == 0),
                            stop=(k == len(K1) - 1),
                        )
                    h = hpool.tile([128, NB], F32R, tag=f"h1m{m}")
                    elu_plus1(ps, b1t[:, m : m + 1], h, relu_on_act=(m < 2))
                    hs.append(h)
                h1s[t] = hs

            def emit_l2(t):
                hin = h1s.pop(t)
                hs = []
                for m in range(2):
                    ps = pp.tile([128, NB], F32, tag=f"ps2m{m}")
                    for k in range(4):
                        nc.tensor.matmul(
                            ps[:],
                            w2[k][:, m * 128 : (m + 1) * 128],
                            hin[k][:],
                            start=(k == 0),
                            stop=(k == 3),
                        )
                    h = hpool.tile([128, NB], F32R, tag=f"h2m{m}")
                    elu_plus1(ps, b2t[:, m : m + 1], h, relu_on_act=(m == 0))
                    hs.append(h)
                h2s[t] = hs

            def emit_l3(t):
                hin = h2s.pop(t)
                ps = pp.tile([128, NB], F32, tag="ps3")
                for k in range(2):
                    nc.tensor.matmul(
                        ps[:],
                        w3[k][:, :],
                        hin[k][:],
                        start=(k == 0),
                        stop=(k == 1),
                    )
                h = hpool.tile([128, NB], F32R, tag="h3")
                elu_plus1(ps, b3t[:, 0:1], h, relu_on_act=False)
                h3s[t] = h

            def emit_l4(t):
                hin = h3s.pop(t)
                c0 = t * NB
                ps = pp.tile([DIMS[4], NB], F32, tag="ps4")
                nc.tensor.matmul(
                    ps[:],
                    w4[:, :],
                    hin[:],
                    start=True,
                    stop=True,
                )
                ot = opool.tile([DIMS[4], NB], F32, tag="yt")
                nc.scalar.activation(ot[:], ps[:], AF.Identity, bias=b4t[:, 0:1])
                nc.sync.dma_start(yt_d[:, c0 : c0 + NB], ot[:])

            # ---- software-pipelined emission, 4 deep ----
            emit_dma_x(0)
            for t in range(NCHUNK + 3):
                if t < NCHUNK:
                    if t + 1 < NCHUNK:
                        emit_dma_x(t + 1)
                    emit_l1(t)
                if 0 <= t - 1 < NCHUNK:
                    emit_l2(t - 1)
                if 0 <= t - 2 < NCHUNK:
                    emit_l3(t - 2)
                if 0 <= t - 3 < NCHUNK:
                    emit_l4(t - 3)

    nc.compile()
    _CACHE["nc"] = nc
    return nc


def _tf32_rn(a):
    """Round fp32 array to TF32 (10-bit mantissa), round-to-nearest-even.
    fp32r matmul operands must be pre-rounded; the PE uses TF32 internally."""
    u = np.ascontiguousarray(a).view(np.uint32)
    r = (u + np.uint32(0x0FFF) + ((u >> np.uint32(13)) & np.uint32(1))) & np.uint32(
        0xFFFFE000
    )
    return r.view(np.float32)


def _prep_in_maps(x, W1, b1, W2, b2, W3, b3, W4, b4):
    x = np.asarray(x, dtype=np.float32)
    shared = {
        "w1t": _tf32_rn(np.asarray(W1, np.float32).T),
        "w2t": _tf32_rn(np.asarray(W2, np.float32).T),
        "w3t": _tf32_rn(np.asarray(W3, np.float32).T),
        "w4t": _tf32_rn(np.asarray(W4, np.float32).T),
    }
    # bias adjustments: downstream layers see h' = elu(h)+1, so subtract W @ 1.
    b1a = np.asarray(b1, np.float32)
    b2a = np.asarray(b2, np.float32) - np.asarray(W2, np.float32).sum(axis=1)
    b3a = np.asarray(b3, np.float32) - np.asarray(W3, np.float32).sum(axis=1)
    b4a = np.asarray(b4, np.float32) - np.asarray(W4, np.float32).sum(axis=1)
    shared["b1c"] = np.ascontiguousarray(b1a.reshape(4, 128).T)
    shared["b2c"] = np.ascontiguousarray(b2a.reshape(2, 128).T)
    shared["b3c"] = np.ascontiguousarray(b3a.reshape(1, 128).T)
    shared["b4c"] = np.ascontiguousarray(b4a.reshape(DIMS[4], 1))

    in_maps = []
    for i in range(N_CORES):
        xs = _tf32_rn(x[i * BS : (i + 1) * BS].T)  # [795, 8192]
        in_maps.append({**shared, "xt": xs})
    return in_maps


def _run(in_maps, trace=False, **kw):
    nc = _build_nc()
    return run_bass_kernel_spmd(nc, in_maps, list(range(N_CORES)), trace=trace, **kw)


def kernel(x, W1, b1, W2, b2, W3, b3, W4, b4):
    in_maps = _prep_in_maps(x, W1, b1, W2, b2, W3, b3, W4, b4)
    res = _run(in_maps).results
    y = np.empty((B, DIMS[4]), dtype=np.float32)
    for i in range(N_CORES):
        y[i * BS : (i + 1) * BS, :] = res[i]["yt"].T
    return y
